# revision 17
# baseline (speedup 1.0000x reference)
"""Distributed TRN2 Bass kernel for ActionBackgroundExamplarContrastiveLoss.

Math (see the reference): with feats [F=200000, D=128], idxs [F] in [0,200),
examplars ex [N=200, D=128]:
    sums[n]   = sum_{i: idx_i=n} feats[i]        counts[n] = |{i: idx_i=n}|
    means     = sums / counts
    exn       = ex / max(||ex||_row, 1e-12)
    logits    = (means @ exn.T) / TEMP
    shifted   = logits - rowmax(logits)
    log_prob  = shifted - logsumexp_row(shifted)
    loss      = -(TEMP/BASE_TEMP) * sum(log_prob) / N^2
(The reference's mask2 is the identity, so its masked mean collapses to the
full sum of log_prob over N^2.)

Strategy (data parallel over F, 8 cores):
  * Each core gets F/8 = 25000 feature rows (padded to 25088 = 196*128 with
    out-of-range class ids, which one-hot to zero rows).
  * Per 128-row chunk: DVE builds the one-hot O[p, c] = (idx[p] == c) via
    tensor_scalar(is_equal) against an iota constant; PE accumulates
    sumsT[d, n] += feats_chunk.T @ O  (PSUM [128, 200]) and
    counts[1, n] += ones.T @ O        (PSUM [1, 200]).
  * AllReduce a packed [129, 200] buffer (sumsT + counts) across the 8 cores.
  * Every core redundantly computes the tiny [200, 200] loss epilogue;
    partition-dim broadcasts/transposes of small vectors are done with K=1
    matmuls on the PE.

Host-side prep only reshapes/pads inputs so every DMA is a clean
128-partition linear transfer (feature rows land on partitions).
"""

import numpy as np

import concourse.bass as bass
import concourse.tile as tile
from concourse import bacc, mybir
from concourse import bass_utils

F32 = mybir.dt.float32
BF16 = mybir.dt.bfloat16

N_CORES = 8
F_TOTAL = 200000
F_SHARD = F_TOTAL // N_CORES      # 25000
CHUNK = 128
N_CHUNKS = 196                    # ceil(25000/128)
F_PAD = N_CHUNKS * CHUNK          # 25088
D = 128
N = 200                           # num examplars / classes
TEMP = 0.07
BASE_TEMP = 0.07
GROUP = 14                        # chunks per feats DMA -> 896KB f32 read
N_GROUPS = N_CHUNKS // GROUP      # 14
PAD_CLASS = 255.0                 # one-hot of out-of-range class is all-zero


def _bfly_allreduce(tc, acc, scratch_pool):
    """In-place butterfly all-reduce of contiguous SBUF tile `acc` [128, w]
    across the 8 cores: 3 rounds of XOR-relative remote_dma_broadcast
    (partner = me XOR 2^r), one arrival semaphore per round."""
    nc = tc.nc
    w = acc.shape[1]
    recv = []
    for r in range(3):
        rt = scratch_pool.tile([128, w], acc.dtype, tag=f"bfly_recv{r}")
        recv.append(rt)
    with (
        nc.semaphore() as rsem0,
        nc.semaphore() as rsem1,
        nc.semaphore() as rsem2,
        nc.semaphore() as lsem,
        nc.semaphore() as psem,
        nc.semaphore() as vsem,
    ):
        rsems = [rsem0, rsem1, rsem2]
        with tc.tile_critical():
            for r in range(3):
                delta = 1 << r
                rdests = [None] * 8
                rdests[delta] = (0, delta)
                if r > 0:
                    nc.gpsimd.wait_ge(vsem, r)
                nc.gpsimd.remote_dma_broadcast(
                    recv[r][:],
                    acc[:],
                    remote_sem=rsems[r],
                    local_sem=lsem,
                    rdests=rdests,
                    queue_num=0,
                ).then_inc(psem, 1)
                nc.gpsimd.wait_ge(psem, r + 1)
                nc.gpsimd.trigger_dma(1, queue_num=0)
                nc.vector.wait_ge(rsems[r], 2)
                nc.vector.wait_ge(lsem, 16 * (r + 1))
                nc.vector.tensor_tensor(
                    acc[:], acc[:], recv[r][:], mybir.AluOpType.add
                ).then_inc(vsem, 1)
            nc.gpsimd.wait_ge(vsem, 3)
    return acc


def build_kernel():
    nc = bacc.Bacc(
        "TRN2",
        target_bir_lowering=False,
        debug=False,
        enable_asserts=True,
        num_devices=N_CORES,
    )

    # Host-prepped per-core inputs (see kernel() below for layouts).
    feats = nc.dram_tensor("feats", [128, F_PAD], F32, kind="ExternalInput")
    idxs = nc.dram_tensor("idxs", [128, N_CHUNKS], F32, kind="ExternalInput")
    ex_t = nc.dram_tensor("ex_t", [D, N], F32, kind="ExternalInput")
    out = nc.dram_tensor("out", [1, 1], F32, kind="ExternalOutput")

    import ml_dtypes

    iota_np = np.tile(np.arange(N, dtype=ml_dtypes.bfloat16), (128, 1))
    iota_c = nc.inline_tensor(iota_np, "iota_c")                  # [128, 200] bf16
    ones_c = nc.inline_tensor(np.ones((128, 128), np.float32), "ones_c")
    ones_bf_c = nc.inline_tensor(np.ones((128, 128), ml_dtypes.bfloat16), "ones_bf_c")

    with tile.TileContext(nc) as tc:
        with (
            tc.tile_pool(name="singles", bufs=1) as singles,
            tc.tile_pool(name="fstream", bufs=6) as fstream,
            tc.tile_pool(name="onehot", bufs=8) as onehot_pool,
            tc.tile_pool(name="acc_psum", bufs=1, space="PSUM") as acc_psum,
            tc.tile_pool(name="epi_psum", bufs=2, space="PSUM") as epi_psum,
            tc.tile_pool(name="epi", bufs=1) as epi,
            tc.tile_pool(name="dram", bufs=1, space="DRAM") as dram,
        ):
            # --- constants and small inputs into SBUF ---
            iota_sb = singles.tile([128, N], BF16, tag="iota")
            nc.scalar.dma_start(iota_sb[:], iota_c.ap()[:])
            ones_sb = singles.tile([128, 128], F32, tag="ones")
            nc.scalar.dma_start(ones_sb[:], ones_c.ap()[:])
            ones_bf_sb = singles.tile([128, 128], BF16, tag="ones_bf")
            nc.scalar.dma_start(ones_bf_sb[:], ones_bf_c.ap()[:])
            idx_sb = singles.tile([128, N_CHUNKS], F32, tag="idx")
            nc.scalar.dma_start(idx_sb[:], idxs.ap()[:])
            ext_sb = singles.tile([D, N], F32, tag="ext")
            nc.scalar.dma_start(ext_sb[:], ex_t.ap()[:])

            # --- main loop: segment sums + counts into PSUM ---
            ps_sums = acc_psum.tile([128, N], F32, tag="ps_sums")   # sums.T
            ps_cnt = acc_psum.tile([128, N], F32, tag="ps_cnt")     # every row = counts
            for g in range(N_GROUPS):
                ftile_f = fstream.tile([128, GROUP * CHUNK], F32, tag="ftile_f")
                nc.sync.dma_start(
                    ftile_f[:],
                    feats.ap()[:, g * GROUP * CHUNK : (g + 1) * GROUP * CHUNK],
                )
                ftile = fstream.tile([128, GROUP * CHUNK], BF16, tag="ftile")
                nc.scalar.activation(
                    ftile[:], ftile_f[:], mybir.ActivationFunctionType.Copy
                )
                for j in range(GROUP):
                    k = g * GROUP + j
                    oh = onehot_pool.tile([128, N], BF16, tag="oh")
                    nc.vector.tensor_scalar(
                        oh[:],
                        iota_sb[:],
                        idx_sb[:, k : k + 1],
                        None,
                        mybir.AluOpType.is_equal,
                    )
                    nc.tensor.matmul(
                        ps_sums[:],
                        ftile[:, j * CHUNK : (j + 1) * CHUNK],
                        oh[:],
                        start=(k == 0),
                        stop=(k == N_CHUNKS - 1),
                    )
                    nc.tensor.matmul(
                        ps_cnt[:],
                        ones_bf_sb[:],
                        oh[:],
                        start=(k == 0),
                        stop=(k == N_CHUNKS - 1),
                    )

            # --- pack partials [sums.T | countsT(0:128) | countsT(128:200)] ---
            cnt_row_sb = epi.tile([1, N], F32, tag="cnt_row_sb")
            nc.scalar.activation(
                cnt_row_sb[:], ps_cnt[0:1, :], mybir.ActivationFunctionType.Copy
            )
            ps_c1 = epi_psum.tile([128, 1], F32, tag="epi_mm")
            nc.tensor.matmul(
                ps_c1[:], cnt_row_sb[:, 0:128], ones_sb[0:1, 0:1],
                start=True, stop=True,
            )
            ps_c2 = epi_psum.tile([72, 1], F32, tag="epi_mm")
            nc.tensor.matmul(
                ps_c2[:], cnt_row_sb[:, 128:N], ones_sb[0:1, 0:1],
                start=True, stop=True,
            )
            acc = epi.tile([128, N + 2], F32, tag="acc")
            nc.scalar.activation(
                acc[:, 0:N], ps_sums[:], mybir.ActivationFunctionType.Copy
            )
            nc.scalar.activation(
                acc[:, N : N + 1], ps_c1[:], mybir.ActivationFunctionType.Copy
            )
            nc.vector.memset(acc[:, N + 1 : N + 2], 0.0)
            nc.scalar.activation(
                acc[0:72, N + 1 : N + 2], ps_c2[:], mybir.ActivationFunctionType.Copy
            )

            _bfly_allreduce(tc, acc, epi)
            sums_sb = acc[:, 0:N]

            # --- epilogue (tiny, replicated on every core) ---
            # normalize examplars: column norms of ex_t via ones.T @ (ex_t^2)
            ex2 = epi.tile([D, N], F32, tag="ex2")
            nc.vector.tensor_tensor(ex2[:], ext_sb[:], ext_sb[:], mybir.AluOpType.mult)
            ps_n2 = epi_psum.tile([1, N], F32, tag="epi_mm")
            nc.tensor.matmul(ps_n2[:], ones_sb[:, 0:1], ex2[:], start=True, stop=True)
            nrm = epi.tile([1, N], F32, tag="nrm")
            nc.scalar.sqrt(nrm[:], ps_n2[:])
            nc.vector.tensor_scalar_max(nrm[:], nrm[:], 1e-12)
            inv_nrm = epi.tile([1, N], F32, tag="inv_nrm")
            nc.vector.reciprocal(inv_nrm[:], nrm[:])
            # broadcast inv_nrm across partitions: [128,1] ones (x) [1,200]
            ps_bc = epi_psum.tile([128, N], F32, tag="epi_mm")
            nc.tensor.matmul(
                ps_bc[:], ones_sb[0:1, :], inv_nrm[:], start=True, stop=True
            )
            # rhs for logits: [exn | rowsum(exn)]  ([128, 201])
            exn = epi.tile([D, N + 1], F32, tag="exn")
            nc.vector.tensor_tensor(
                exn[:, 0:N], ext_sb[:], ps_bc[:], mybir.AluOpType.mult
            )
            nc.vector.reduce_sum(
                exn[:, N : N + 1], exn[:, 0:N], axis=mybir.AxisListType.X
            )

            # counts -> 1/(TEMP*count) per-partition scales (from acc columns)
            rc1 = epi.tile([128, 1], F32, tag="rc1")
            nc.vector.tensor_scalar_mul(rc1[:], acc[:, N : N + 1], TEMP)
            nc.vector.reciprocal(rc1[:], rc1[:])
            rc2 = epi.tile([72, 1], F32, tag="rc2")
            nc.vector.tensor_scalar_mul(rc2[:], acc[0:72, N + 1 : N + 2], TEMP)
            nc.vector.reciprocal(rc2[:], rc2[:])

            # logits halves; no explicit row-max (|logits| <= ~45, exp is safe)
            total_ps = epi_psum.tile([1, 1], F32, tag="total")
            for half, (m0, m1, rc) in enumerate(((0, 128, rc1), (128, N, rc2))):
                mrows = m1 - m0
                ps_l = epi_psum.tile([mrows, N + 1], F32, tag="ps_l")
                nc.tensor.matmul(
                    ps_l[:], sums_sb[:, m0:m1], exn[:], start=True, stop=True
                )
                # e = exp(raw * rc[p]); se = rowsum(e)   (scale rides the ACT op)
                esh = epi.tile([mrows, N], F32, tag=f"esh{half}")
                se = epi.tile([mrows, 1], F32, tag=f"se{half}")
                nc.scalar.activation(
                    esh[:],
                    ps_l[:, 0:N],
                    mybir.ActivationFunctionType.Exp,
                    scale=rc[:],
                    accum_out=se[:],
                )
                lse = epi.tile([mrows, 1], F32, tag=f"lse{half}")
                nc.scalar.activation(lse[:], se[:], mybir.ActivationFunctionType.Ln)
                u = epi.tile([mrows, 1], F32, tag=f"u{half}")
                nc.vector.tensor_scalar_mul(u[:], lse[:], -float(N))
                # rowtotal = rowsum(logits) - N*lse = ps_l[:, N]*rc - N*lse
                rtot = epi.tile([mrows, 1], F32, tag=f"rtot{half}")
                nc.vector.scalar_tensor_tensor(
                    rtot[:],
                    ps_l[:, N : N + 1],
                    rc[:],
                    u[:],
                    mybir.AluOpType.mult,
                    mybir.AluOpType.add,
                )
                nc.tensor.matmul(
                    total_ps[:],
                    ones_sb[0:mrows, 0:1],
                    rtot[:],
                    start=(half == 0),
                    stop=(half == 1),
                )

            out_sb = epi.tile([1, 1], F32, tag="out_sb")
            scale = -(TEMP / BASE_TEMP) / float(N * N)
            nc.vector.tensor_scalar_mul(out_sb[:], total_ps[:], scale)
            nc.sync.dma_start(out.ap()[:], out_sb[:])

    nc.compile()
    return nc


def _prep_in_maps(feats, idxs, ex):
    import ml_dtypes

    feats = np.ascontiguousarray(np.asarray(feats, dtype=np.float32))
    idxs_f = np.asarray(idxs).astype(np.float32)
    ex_t = np.ascontiguousarray(np.asarray(ex, dtype=np.float32).T)  # [128, 200]
    in_maps = []
    for c in range(N_CORES):
        fs = feats[c * F_SHARD : (c + 1) * F_SHARD]
        fs = np.concatenate([fs, np.zeros((F_PAD - F_SHARD, D), np.float32)], axis=0)
        # rows -> partitions: [196, 128, 128] -> [128, 196*128]; each partition
        # owns a contiguous 100KB strip so feats DMAs are linear.
        fs = np.ascontiguousarray(
            fs.reshape(N_CHUNKS, CHUNK, D).transpose(1, 0, 2).reshape(128, F_PAD)
        )
        ii = idxs_f[c * F_SHARD : (c + 1) * F_SHARD]
        ii = np.concatenate(
            [ii, np.full((F_PAD - F_SHARD,), PAD_CLASS, np.float32)], axis=0
        )
        ii = np.ascontiguousarray(ii.reshape(N_CHUNKS, CHUNK).T)  # [128, 196]
        in_maps.append({"feats": fs, "idxs": ii, "ex_t": ex_t})
    return in_maps


_NC_CACHE = None


def _get_nc():
    global _NC_CACHE
    if _NC_CACHE is None:
        _NC_CACHE = build_kernel()
    return _NC_CACHE


def run(feats, idxs, ex, trace=False, **kwargs):
    nc = _get_nc()
    in_maps = _prep_in_maps(feats, idxs, ex)
    res = bass_utils.run_bass_kernel_spmd(
        nc, in_maps, core_ids=list(range(N_CORES)), trace=trace, **kwargs
    )
    val = np.float32(np.asarray(res.results[0]["out"]).reshape(()))
    return val, res


def kernel(
    actionbackground_features_actionframes, action_idxs_actionframes, examplars
):
    val, _ = run(
        actionbackground_features_actionframes,
        action_idxs_actionframes,
        examplars,
    )
    return np.asarray(val, dtype=np.float32).reshape(())


# revision 18
# speedup vs baseline: 49.1605x; 49.1605x over previous
"""Distributed TRN2 Bass kernel for ActionBackgroundExamplarContrastiveLoss.

Math (see the reference): with feats [F=200000, D=128], idxs [F] in [0,200),
examplars ex [N=200, D=128]:
    sums[n]   = sum_{i: idx_i=n} feats[i]        counts[n] = |{i: idx_i=n}|
    means     = sums / counts
    exn       = ex / max(||ex||_row, 1e-12)
    logits    = (means @ exn.T) / TEMP
    shifted   = logits - rowmax(logits)
    log_prob  = shifted - logsumexp_row(shifted)
    loss      = -(TEMP/BASE_TEMP) * sum(log_prob) / N^2
(The reference's mask2 is the identity, so its masked mean collapses to the
full sum of log_prob over N^2.)

Strategy (data parallel over F, 8 cores):
  * Each core gets F/8 = 25000 feature rows (padded to 25088 = 196*128 with
    out-of-range class ids, which one-hot to zero rows).
  * Per 128-row chunk: DVE builds the one-hot O[p, c] = (idx[p] == c) via
    tensor_scalar(is_equal) against an iota constant; PE accumulates
    sumsT[d, n] += feats_chunk.T @ O  (PSUM [128, 200]) and
    counts[1, n] += ones.T @ O        (PSUM [1, 200]).
  * AllReduce a packed [129, 200] buffer (sumsT + counts) across the 8 cores.
  * Every core redundantly computes the tiny [200, 200] loss epilogue;
    partition-dim broadcasts/transposes of small vectors are done with K=1
    matmuls on the PE.

Host-side prep only reshapes/pads inputs so every DMA is a clean
128-partition linear transfer (feature rows land on partitions).
"""

import numpy as np

import concourse.bass as bass
import concourse.tile as tile
from concourse import bacc, mybir
from concourse import bass_utils

F32 = mybir.dt.float32
BF16 = mybir.dt.bfloat16

N_CORES = 8
F_TOTAL = 200000
F_SHARD = F_TOTAL // N_CORES      # 25000
CHUNK = 128
N_CHUNKS = 196                    # ceil(25000/128)
F_PAD = N_CHUNKS * CHUNK          # 25088
D = 128
N = 200                           # num examplars / classes
TEMP = 0.07
BASE_TEMP = 0.07
GROUP = 14                        # chunks per feats DMA -> 896KB f32 read
N_GROUPS = N_CHUNKS // GROUP      # 14
PAD_CLASS = 255.0                 # one-hot of out-of-range class is all-zero


def _bfly_allreduce(tc, acc, scratch_pool):
    """In-place butterfly all-reduce of contiguous SBUF tile `acc` [128, w]
    across the 8 cores: 3 rounds of XOR-relative remote_dma_broadcast
    (partner = me XOR 2^r), one arrival semaphore per round."""
    nc = tc.nc
    w = acc.shape[1]
    recv = []
    for r in range(3):
        rt = scratch_pool.tile([128, w], acc.dtype, tag=f"bfly_recv{r}")
        recv.append(rt)
    with (
        nc.semaphore() as rsem0,
        nc.semaphore() as rsem1,
        nc.semaphore() as rsem2,
        nc.semaphore() as lsem,
        nc.semaphore() as psem,
        nc.semaphore() as vsem,
    ):
        rsems = [rsem0, rsem1, rsem2]
        with tc.tile_critical():
            for r in range(3):
                delta = 1 << r
                rdests = [None] * 8
                rdests[delta] = (0, delta)
                if r > 0:
                    nc.gpsimd.wait_ge(vsem, r)
                nc.gpsimd.remote_dma_broadcast(
                    recv[r][:],
                    acc[:],
                    remote_sem=rsems[r],
                    local_sem=lsem,
                    rdests=rdests,
                    queue_num=0,
                ).then_inc(psem, 1)
                nc.gpsimd.wait_ge(psem, r + 1)
                nc.gpsimd.trigger_dma(1, queue_num=0)
                nc.vector.wait_ge(rsems[r], 2)
                nc.vector.wait_ge(lsem, 16 * (r + 1))
                nc.vector.tensor_tensor(
                    acc[:], acc[:], recv[r][:], mybir.AluOpType.add
                ).then_inc(vsem, 1)
            nc.gpsimd.wait_ge(vsem, 3)
    return acc


def build_kernel():
    nc = bacc.Bacc(
        "TRN2",
        target_bir_lowering=False,
        debug=False,
        enable_asserts=True,
        num_devices=N_CORES,
    )

    # Host-prepped per-core inputs (see kernel() below for layouts).
    feats = nc.dram_tensor("feats", [128, F_PAD], F32, kind="ExternalInput")
    idxs = nc.dram_tensor("idxs", [128, N_CHUNKS], F32, kind="ExternalInput")
    ex_t = nc.dram_tensor("ex_t", [D, N], F32, kind="ExternalInput")
    out = nc.dram_tensor("out", [1, 1], F32, kind="ExternalOutput")

    import ml_dtypes

    iota_np = np.tile(np.arange(N, dtype=ml_dtypes.bfloat16), (128, 1))
    iota_c = nc.inline_tensor(iota_np, "iota_c")                  # [128, 200] bf16
    ones_c = nc.inline_tensor(np.ones((128, 128), np.float32), "ones_c")
    ones_bf_c = nc.inline_tensor(np.ones((128, 128), ml_dtypes.bfloat16), "ones_bf_c")

    with tile.TileContext(nc) as tc:
        with (
            tc.tile_pool(name="singles", bufs=1) as singles,
            tc.tile_pool(name="fstream", bufs=6) as fstream,
            tc.tile_pool(name="onehot", bufs=8) as onehot_pool,
            tc.tile_pool(name="acc_psum", bufs=1, space="PSUM") as acc_psum,
            tc.tile_pool(name="epi_psum", bufs=2, space="PSUM") as epi_psum,
            tc.tile_pool(name="epi", bufs=1) as epi,
            tc.tile_pool(name="dram", bufs=1, space="DRAM") as dram,
        ):
            # --- constants and small inputs into SBUF ---
            iota_sb = singles.tile([128, N], BF16, tag="iota")
            nc.gpsimd.dma_start(iota_sb[:], iota_c.ap()[:])
            ones_sb = singles.tile([128, 128], F32, tag="ones")
            nc.gpsimd.dma_start(ones_sb[:], ones_c.ap()[:])
            ones_bf_sb = singles.tile([128, 128], BF16, tag="ones_bf")
            nc.gpsimd.dma_start(ones_bf_sb[:], ones_bf_c.ap()[:])
            idx_sb = singles.tile([128, N_CHUNKS], F32, tag="idx")
            nc.gpsimd.dma_start(idx_sb[:], idxs.ap()[:])
            ext_sb = singles.tile([D, N], F32, tag="ext")
            nc.gpsimd.dma_start(ext_sb[:], ex_t.ap()[:])

            # --- main loop: segment sums + counts into PSUM ---
            ps_sums = acc_psum.tile([128, N], F32, tag="ps_sums")   # sums.T
            ps_cnt = acc_psum.tile([128, N], F32, tag="ps_cnt")     # every row = counts
            for g in range(N_GROUPS):
                ftile = fstream.tile([128, GROUP * CHUNK], BF16, tag="ftile")
                # SWDGE casts f32 -> bf16 in flight (HBM still reads full f32)
                nc.gpsimd.dma_start(
                    ftile[:], feats.ap()[:, g * GROUP * CHUNK : (g + 1) * GROUP * CHUNK]
                )
                for j in range(GROUP):
                    k = g * GROUP + j
                    oh = onehot_pool.tile([128, N], BF16, tag="oh")
                    nc.vector.tensor_scalar(
                        oh[:],
                        iota_sb[:],
                        idx_sb[:, k : k + 1],
                        None,
                        mybir.AluOpType.is_equal,
                    )
                    nc.tensor.matmul(
                        ps_sums[:],
                        ftile[:, j * CHUNK : (j + 1) * CHUNK],
                        oh[:],
                        start=(k == 0),
                        stop=(k == N_CHUNKS - 1),
                    )
                    nc.tensor.matmul(
                        ps_cnt[:],
                        ones_bf_sb[:],
                        oh[:],
                        start=(k == 0),
                        stop=(k == N_CHUNKS - 1),
                    )

            # --- pack partials [sums.T | countsT(0:128) | countsT(128:200)] ---
            cnt_row_sb = epi.tile([1, N], F32, tag="cnt_row_sb")
            nc.scalar.activation(
                cnt_row_sb[:], ps_cnt[0:1, :], mybir.ActivationFunctionType.Copy
            )
            ps_c1 = epi_psum.tile([128, 1], F32, tag="epi_mm")
            nc.tensor.matmul(
                ps_c1[:], cnt_row_sb[:, 0:128], ones_sb[0:1, 0:1],
                start=True, stop=True,
            )
            ps_c2 = epi_psum.tile([72, 1], F32, tag="epi_mm")
            nc.tensor.matmul(
                ps_c2[:], cnt_row_sb[:, 128:N], ones_sb[0:1, 0:1],
                start=True, stop=True,
            )
            acc = epi.tile([128, N + 2], F32, tag="acc")
            nc.scalar.activation(
                acc[:, 0:N], ps_sums[:], mybir.ActivationFunctionType.Copy
            )
            nc.scalar.activation(
                acc[:, N : N + 1], ps_c1[:], mybir.ActivationFunctionType.Copy
            )
            nc.vector.memset(acc[:, N + 1 : N + 2], 0.0)
            nc.scalar.activation(
                acc[0:72, N + 1 : N + 2], ps_c2[:], mybir.ActivationFunctionType.Copy
            )

            ar_in = dram.tile([128, N + 2], F32, tag="ar_in")
            ar_out = dram.tile([128, N + 2], F32, tag="ar_out")
            nc.sync.dma_start(ar_in[:], acc[:])
            nc.gpsimd.collective_compute(
                "AllReduce",
                mybir.AluOpType.add,
                replica_groups=[list(range(N_CORES))],
                ins=[ar_in.opt()],
                outs=[ar_out.opt()],
            )
            nc.sync.dma_start(acc[:], ar_out[:])
            sums_sb = acc[:, 0:N]

            # --- epilogue (tiny, replicated on every core) ---
            # normalize examplars: column norms of ex_t via ones.T @ (ex_t^2)
            ex2 = epi.tile([D, N], F32, tag="ex2")
            nc.vector.tensor_tensor(ex2[:], ext_sb[:], ext_sb[:], mybir.AluOpType.mult)
            ps_n2 = epi_psum.tile([1, N], F32, tag="epi_mm")
            nc.tensor.matmul(ps_n2[:], ones_sb[:, 0:1], ex2[:], start=True, stop=True)
            nrm = epi.tile([1, N], F32, tag="nrm")
            nc.scalar.sqrt(nrm[:], ps_n2[:])
            nc.vector.tensor_scalar_max(nrm[:], nrm[:], 1e-12)
            inv_nrm = epi.tile([1, N], F32, tag="inv_nrm")
            nc.vector.reciprocal(inv_nrm[:], nrm[:])
            # broadcast inv_nrm across partitions: [128,1] ones (x) [1,200]
            ps_bc = epi_psum.tile([128, N], F32, tag="epi_mm")
            nc.tensor.matmul(
                ps_bc[:], ones_sb[0:1, :], inv_nrm[:], start=True, stop=True
            )
            # rhs for logits: [exn | rowsum(exn)]  ([128, 201])
            exn = epi.tile([D, N + 1], F32, tag="exn")
            nc.vector.tensor_tensor(
                exn[:, 0:N], ext_sb[:], ps_bc[:], mybir.AluOpType.mult
            )
            nc.vector.reduce_sum(
                exn[:, N : N + 1], exn[:, 0:N], axis=mybir.AxisListType.X
            )

            # counts -> 1/(TEMP*count) per-partition scales (from acc columns)
            rc1 = epi.tile([128, 1], F32, tag="rc1")
            nc.vector.tensor_scalar_mul(rc1[:], acc[:, N : N + 1], TEMP)
            nc.vector.reciprocal(rc1[:], rc1[:])
            rc2 = epi.tile([72, 1], F32, tag="rc2")
            nc.vector.tensor_scalar_mul(rc2[:], acc[0:72, N + 1 : N + 2], TEMP)
            nc.vector.reciprocal(rc2[:], rc2[:])

            # logits halves; no explicit row-max (|logits| <= ~45, exp is safe)
            total_ps = epi_psum.tile([1, 1], F32, tag="total")
            for half, (m0, m1, rc) in enumerate(((0, 128, rc1), (128, N, rc2))):
                mrows = m1 - m0
                ps_l = epi_psum.tile([mrows, N + 1], F32, tag="ps_l")
                nc.tensor.matmul(
                    ps_l[:], sums_sb[:, m0:m1], exn[:], start=True, stop=True
                )
                # e = exp(raw * rc[p]); se = rowsum(e)   (scale rides the ACT op)
                esh = epi.tile([mrows, N], F32, tag=f"esh{half}")
                se = epi.tile([mrows, 1], F32, tag=f"se{half}")
                nc.scalar.activation(
                    esh[:],
                    ps_l[:, 0:N],
                    mybir.ActivationFunctionType.Exp,
                    scale=rc[:],
                    accum_out=se[:],
                )
                lse = epi.tile([mrows, 1], F32, tag=f"lse{half}")
                nc.scalar.activation(lse[:], se[:], mybir.ActivationFunctionType.Ln)
                u = epi.tile([mrows, 1], F32, tag=f"u{half}")
                nc.vector.tensor_scalar_mul(u[:], lse[:], -float(N))
                # rowtotal = rowsum(logits) - N*lse = ps_l[:, N]*rc - N*lse
                rtot = epi.tile([mrows, 1], F32, tag=f"rtot{half}")
                nc.vector.scalar_tensor_tensor(
                    rtot[:],
                    ps_l[:, N : N + 1],
                    rc[:],
                    u[:],
                    mybir.AluOpType.mult,
                    mybir.AluOpType.add,
                )
                nc.tensor.matmul(
                    total_ps[:],
                    ones_sb[0:mrows, 0:1],
                    rtot[:],
                    start=(half == 0),
                    stop=(half == 1),
                )

            out_sb = epi.tile([1, 1], F32, tag="out_sb")
            scale = -(TEMP / BASE_TEMP) / float(N * N)
            nc.vector.tensor_scalar_mul(out_sb[:], total_ps[:], scale)
            nc.sync.dma_start(out.ap()[:], out_sb[:])

    nc.compile()
    return nc


def _prep_in_maps(feats, idxs, ex):
    import ml_dtypes

    feats = np.ascontiguousarray(np.asarray(feats, dtype=np.float32))
    idxs_f = np.asarray(idxs).astype(np.float32)
    ex_t = np.ascontiguousarray(np.asarray(ex, dtype=np.float32).T)  # [128, 200]
    in_maps = []
    for c in range(N_CORES):
        fs = feats[c * F_SHARD : (c + 1) * F_SHARD]
        fs = np.concatenate([fs, np.zeros((F_PAD - F_SHARD, D), np.float32)], axis=0)
        # rows -> partitions: [196, 128, 128] -> [128, 196*128]; each partition
        # owns a contiguous 100KB strip so feats DMAs are linear.
        fs = np.ascontiguousarray(
            fs.reshape(N_CHUNKS, CHUNK, D).transpose(1, 0, 2).reshape(128, F_PAD)
        )
        ii = idxs_f[c * F_SHARD : (c + 1) * F_SHARD]
        ii = np.concatenate(
            [ii, np.full((F_PAD - F_SHARD,), PAD_CLASS, np.float32)], axis=0
        )
        ii = np.ascontiguousarray(ii.reshape(N_CHUNKS, CHUNK).T)  # [128, 196]
        in_maps.append({"feats": fs, "idxs": ii, "ex_t": ex_t})
    return in_maps


_NC_CACHE = None


def _get_nc():
    global _NC_CACHE
    if _NC_CACHE is None:
        _NC_CACHE = build_kernel()
    return _NC_CACHE


def run(feats, idxs, ex, trace=False, **kwargs):
    nc = _get_nc()
    in_maps = _prep_in_maps(feats, idxs, ex)
    res = bass_utils.run_bass_kernel_spmd(
        nc, in_maps, core_ids=list(range(N_CORES)), trace=trace, **kwargs
    )
    val = np.float32(np.asarray(res.results[0]["out"]).reshape(()))
    return val, res


def kernel(
    actionbackground_features_actionframes, action_idxs_actionframes, examplars
):
    val, _ = run(
        actionbackground_features_actionframes,
        action_idxs_actionframes,
        examplars,
    )
    return np.asarray(val, dtype=np.float32).reshape(())
